# revision 14
# baseline (speedup 1.0000x reference)
"""Trainium2 Bass kernel for nn_AlternativeIdeaModel (gnn_message_passing).

SPMD over 8 NeuronCores, row-sharding spots (625/core) and V rows (2500/core).

U softmax [5000, 20000] dominates (memory regime): host pre-tiles U into
chunk-contiguous DRAM [NRT, NCH, 125, 2500]; the 50MB in-stream and the 25MB
bf16 A out-stream both run on the GpSimd SWDGE queue (fans each DMA across all
16 SDMA engines; the HWDGE rings cap at ~130GB/s for these shapes). exp with
free-dim accumulate on ScalarE, reciprocal+scale on VectorE. Outs for tile t
are emitted after tile t+1's ins so the single SWDGE queue never head-of-line
blocks on the softmax barrier.

GCN uses a dense normalized adjacency (host-built from edge_index), bf16,
column-sharded [5000, 625] and SBUF-resident; per layer: local xw = x @ Wl,
AllGather (bf16), transposed aggregation matmuls on TensorE. V softmax +
TensorE ones-matmul column sum + AllReduce give C*p for M_rec; the decoder
runs replicated. Collectives are emitted between softmax tiles so their queue
position matches when their inputs are ready. All other DMA traffic lives on
the Sync HWDGE ring, ordered by expected readiness.
"""

import os
import sys

sys.path.insert(0, "/opt/trn_rl_repo")

import numpy as np

S, C, G, D, K, HE, HD = 5000, 20000, 2000, 32, 20, 64, 256
G_SC = 2000
NCORES = 8
SSH = S // NCORES          # 625
CSH = C // NCORES          # 2500
RT = 125
NRT = SSH // RT            # 5
CCH = 2500
NCH = C // CCH             # 8
QW = 5000                  # quarter width (2 chunks)
NQ = C // QW               # 4 quarters per row tile
KT1 = (G + 127) // 128     # 16
KTA = (S + 127) // 128     # 40
NVT = CSH // RT            # 20

A_FP16 = True

_NC_CACHE = {}
LAST_RESULT = None


def _build_nc():
    if "nc" in _NC_CACHE:
        return _NC_CACHE["nc"]
    from concourse import bacc, tile, mybir

    f32 = mybir.dt.float32
    bf16 = mybir.dt.bfloat16
    fp16 = mybir.dt.float16
    dtA = fp16 if A_FP16 else f32
    dtU = fp16
    AF = mybir.ActivationFunctionType
    AX = mybir.AxisListType

    nc = bacc.Bacc("TRN2", target_bir_lowering=False, debug=False,
                   num_devices=NCORES)

    U = nc.dram_tensor("U_sh", [NRT, NCH, RT, CCH], dtU,
                       kind="ExternalInput").ap()
    zT = nc.dram_tensor("zT_sh", [G, SSH], bf16, kind="ExternalInput").ap()
    W1 = nc.dram_tensor("W1", [G, HE], bf16, kind="ExternalInput").ap()
    b1 = nc.dram_tensor("b1", [HE, 1], f32, kind="ExternalInput").ap()
    Ah = nc.dram_tensor("AhT_sh", [S, SSH], bf16, kind="ExternalInput").ap()
    W2 = nc.dram_tensor("W2", [HE, D], f32, kind="ExternalInput").ap()
    b2 = nc.dram_tensor("b2", [D, 1], f32, kind="ExternalInput").ap()
    V = nc.dram_tensor("V_sh", [CSH, K], f32, kind="ExternalInput").ap()
    FT = nc.dram_tensor("FT", [D, K], f32, kind="ExternalInput").ap()
    Wd1 = nc.dram_tensor("Wd1", [D, HD], f32, kind="ExternalInput").ap()
    bd1 = nc.dram_tensor("bd1", [128, 2], f32, kind="ExternalInput").ap()
    Wd2b = nc.dram_tensor("Wd2b", [HD + 1, G_SC], bf16,
                          kind="ExternalInput").ap()

    A_out = nc.dram_tensor("A_sh", [NRT, RT, C], dtA,
                           kind="ExternalOutput").ap()
    B_out = nc.dram_tensor("B_sh", [RT, NVT, K], f32,
                           kind="ExternalOutput").ap()
    hT_out = nc.dram_tensor("hT_sh", [D, SSH], f32, kind="ExternalOutput").ap()
    M_out = nc.dram_tensor("M_sh", [K, G_SC], f32, kind="ExternalOutput").ap()

    RG = [list(range(NCORES))]

    with tile.TileContext(nc) as tc:
        with (
            tc.tile_pool(name="const", bufs=1) as constp,
            tc.tile_pool(name="ahat", bufs=KTA) as ahatp,
            tc.tile_pool(name="gcn", bufs=1) as gcnp,
            tc.tile_pool(name="zt", bufs=2) as ztp,
            tc.tile_pool(name="wd2", bufs=2) as wd2p,
            tc.tile_pool(name="uin", bufs=8) as uinp,
            tc.tile_pool(name="ebuf", bufs=7) as ebufp,
            tc.tile_pool(name="acc", bufs=2) as accp,
            tc.tile_pool(name="mrec", bufs=1) as mrecp,
            tc.tile_pool(name="psAgg", bufs=1, space="PSUM") as psAgg,
            tc.tile_pool(name="psAggB", bufs=1, space="PSUM") as psAggB,
            tc.tile_pool(name="psGen", bufs=5, space="PSUM") as psGen,
            tc.tile_pool(name="dram", bufs=1, space="DRAM") as dramp,
        ):
            # ---------- helpers for the softmax tiles ----------
            tiles_state = {}

            def soft_ins(t, mid=None):
                accs = accp.tile([RT, NCH], f32, tag="accs", name=f"acc{t}")
                eq = [ebufp.tile([RT, QW], dtA, tag="eq", name=f"e{t}_{q}")
                      for q in range(NQ)]
                for c in range(NCH):
                    if c == NCH // 2 and mid is not None:
                        mid()
                    uc = uinp.tile([RT, CCH], dtU, tag="uc", name=f"u{t}_{c}")
                    nc.gpsimd.dma_start(uc[:], U[t, c])
                    q, o = divmod(c, NCH // NQ)
                    nc.scalar.activation(eq[q][:, o * CCH:(o + 1) * CCH],
                                         uc[:], AF.Exp,
                                         accum_out=accs[:, c:c + 1])
                ssum = accp.tile([RT, 1], f32, tag="ss", name=f"s{t}")
                nc.vector.reduce_sum(ssum[:], accs[:], axis=AX.X)
                rrec = accp.tile([RT, 1], f32, tag="rr", name=f"r{t}")
                nc.vector.reciprocal(rrec[:], ssum[:])
                tiles_state[t] = (eq, rrec)

            def soft_outs(t):
                eq, rrec = tiles_state.pop(t)
                for q in range(NQ):
                    nc.vector.tensor_scalar_mul(eq[q][:], eq[q][:], rrec[:])
                    nc.gpsimd.dma_start(A_out[t, :, q * QW:(q + 1) * QW],
                                        eq[q][:])

            # ========== B1: layer-1 local matmul path + pure loads =====
            W1s = constp.tile([128, KT1, HE], bf16)
            nc.sync.dma_start(
                W1s[:, :KT1 - 1, :],
                W1[: (KT1 - 1) * 128].rearrange("(k p) n -> p k n", p=128))
            nc.sync.dma_start(W1s[: G - (KT1 - 1) * 128, KT1 - 1, :],
                              W1[(KT1 - 1) * 128:])
            b1s = constp.tile([HE, 1], f32)
            nc.sync.dma_start(b1s[:], b1)
            ones125 = constp.tile([128, 1], f32)
            nc.vector.memset(ones125[:], 1.0)
            ones1 = constp.tile([1, K], bf16)
            nc.vector.memset(ones1[:], 1.0)

            xw1_ps = [psGen.tile([RT, HE], f32, tag="gen", name=f"psxw1_{i}")
                      for i in range(NRT)]
            for k in range(KT1):
                kk = 128 if k < KT1 - 1 else G - (KT1 - 1) * 128
                zt = ztp.tile([128, SSH], bf16, tag="zt", name=f"zt{k}")
                nc.sync.dma_start(zt[:kk, :], zT[k * 128:k * 128 + kk, :])
                for i in range(NRT):
                    nc.tensor.matmul(xw1_ps[i][:],
                                     zt[:kk, i * RT:(i + 1) * RT],
                                     W1s[:kk, k, :],
                                     start=(k == 0), stop=(k == KT1 - 1))
            xw1o = gcnp.tile([RT, NRT, HE], bf16)
            for i in range(NRT):
                nc.vector.tensor_copy(xw1o[:, i, :], xw1_ps[i][:])
            cc1_in = dramp.tile([SSH, HE], bf16)
            cc1_out = dramp.tile([S, HE], bf16)
            nc.sync.dma_start(cc1_in.rearrange("(i p) n -> p i n", p=RT),
                              xw1o[:])
            W2s = constp.tile([HE, D], f32)
            nc.sync.dma_start(W2s[:], W2)
            b2s = constp.tile([D, 1], f32)
            nc.sync.dma_start(b2s[:], b2)
            Vs = gcnp.tile([RT, NVT, K], f32)
            nc.sync.dma_start(Vs[:], V.rearrange("(n p) c -> p n c", p=RT))
            # ========== B2: softmax tile 0 (cc1 trigger mid-tile) ==========
            soft_ins(0, mid=lambda: nc.gpsimd.collective_compute(
                "AllGather", mybir.AluOpType.bypass, replica_groups=RG,
                ins=[cc1_in.opt()], outs=[cc1_out.opt()]))

            # ========== B3: xw1_full readback ==========
            xw1f = gcnp.tile([128, KTA, HE], bf16)
            nc.sync.dma_start(
                xw1f[:, :KTA - 1, :],
                cc1_out[: (KTA - 1) * 128].rearrange("(k p) n -> p k n", p=128))
            nc.sync.dma_start(xw1f[: S - (KTA - 1) * 128, KTA - 1, :],
                              cc1_out[(KTA - 1) * 128:])

            # ========== B4: softmax tile 1, outs 0 ==========
            soft_ins(1)

            # AhatT tiles: pure loads on the fast SWDGE stream (keeps the
            # HWDGE ring free for the latency-critical GCN plumbing)
            ah_tiles = []
            for k in range(KTA):
                r0, r1 = k * 128, min(S, (k + 1) * 128)
                t_ = ahatp.tile([128, SSH], bf16, tag="ah", name=f"ah{k}")
                nc.gpsimd.dma_start(t_[: r1 - r0, :], Ah[r0:r1, :])
                ah_tiles.append(t_)
            soft_outs(0)

            # ========== B5: V softmax + colsum ==========
            Bs = gcnp.tile([RT, NVT, K], f32)
            vsum = gcnp.tile([RT, NVT], f32)
            for n in range(NVT):
                nc.scalar.activation(Bs[:, n, :], Vs[:, n, :], AF.Exp,
                                     accum_out=vsum[:, n:n + 1])
            vrec = gcnp.tile([RT, NVT], f32)
            nc.vector.reciprocal(vrec[:], vsum[:])
            for n in range(NVT):
                nc.vector.tensor_scalar_mul(Bs[:, n, :], Bs[:, n, :],
                                            vrec[:, n:n + 1])
            cs_ps = psGen.tile([K, 1], f32, tag="gen", name="cs_ps")
            for n in range(NVT):
                nc.tensor.matmul(cs_ps[:], Bs[:, n, :], ones125[:RT, :],
                                 start=(n == 0), stop=(n == NVT - 1))
            cs_sb = gcnp.tile([K, 1], f32)
            nc.vector.tensor_copy(cs_sb[:], cs_ps[:])

            # ========== B6: softmax tile 2, outs 1 ==========
            soft_ins(2)
            soft_outs(1)

            # ========== B7: agg1 + h1 + xw2 + cc2 + ccp ==========
            pa = psAgg.tile([HE, 512], f32, tag="agga", name="pa1")
            pb = psAggB.tile([HE, SSH - 512], f32, tag="aggb", name="pb1")
            for k in range(KTA):
                kk = 128 if k < KTA - 1 else S - (KTA - 1) * 128
                nc.tensor.matmul(pa[:], xw1f[:kk, k, :], ah_tiles[k][:kk, :512],
                                 start=(k == 0), stop=(k == KTA - 1))
                nc.tensor.matmul(pb[:], xw1f[:kk, k, :], ah_tiles[k][:kk, 512:],
                                 start=(k == 0), stop=(k == KTA - 1))
            # h1T relu+bias on ScalarE — sits after tile-2 exps in the ACT
            # FIFO, matching when agg1 is actually ready
            h1T = gcnp.tile([HE, SSH], f32)
            nc.scalar.activation(h1T[:, :512], pa[:], AF.Relu, bias=b1s[:])
            nc.scalar.activation(h1T[:, 512:], pb[:], AF.Relu, bias=b1s[:])
            xw2o = gcnp.tile([RT, NRT, D], bf16)
            for i in range(NRT):
                ps = psGen.tile([RT, D], f32, tag="gen", name=f"psxw2_{i}")
                nc.tensor.matmul(ps[:], h1T[:, i * RT:(i + 1) * RT], W2s[:],
                                 start=True, stop=True)
                nc.vector.tensor_copy(xw2o[:, i, :], ps[:])
            cc2_in = dramp.tile([SSH, D], bf16)
            cc2_out = dramp.tile([S, D], bf16)
            nc.sync.dma_start(cc2_in.rearrange("(i p) n -> p i n", p=RT),
                              xw2o[:])
            nc.gpsimd.collective_compute(
                "AllGather", mybir.AluOpType.bypass, replica_groups=RG,
                ins=[cc2_in.opt()], outs=[cc2_out.opt()])
            ccp_in = dramp.tile([K, 1], f32)
            ccp_out = dramp.tile([K, 1], f32)
            nc.gpsimd.dma_start(ccp_in, cs_sb[:])
            nc.gpsimd.collective_compute(
                "AllReduce", mybir.AluOpType.add, replica_groups=RG,
                ins=[ccp_in.opt()], outs=[ccp_out.opt()])
            xw2f = gcnp.tile([128, KTA, D], bf16)
            nc.sync.dma_start(
                xw2f[:, :KTA - 1, :],
                cc2_out[: (KTA - 1) * 128].rearrange("(k p) n -> p k n", p=128))
            nc.sync.dma_start(xw2f[: S - (KTA - 1) * 128, KTA - 1, :],
                              cc2_out[(KTA - 1) * 128:])

            # ========== B8: softmax tile 3, outs 2 ==========
            soft_ins(3)
            soft_outs(2)

            # ========== B9: agg2 matmuls ==========
            pc = psAgg.tile([D, 512], f32, tag="agga", name="pa2")
            pd = psAggB.tile([D, SSH - 512], f32, tag="aggb", name="pb2")
            for k in range(KTA):
                kk = 128 if k < KTA - 1 else S - (KTA - 1) * 128
                nc.tensor.matmul(pc[:], xw2f[:kk, k, :], ah_tiles[k][:kk, :512],
                                 start=(k == 0), stop=(k == KTA - 1))
                nc.tensor.matmul(pd[:], xw2f[:kk, k, :], ah_tiles[k][:kk, 512:],
                                 start=(k == 0), stop=(k == KTA - 1))

            # ========== B10: softmax tile 4, outs 3 ==========
            soft_ins(4)
            soft_outs(3)

            # ========== B11: decoder + M_rec ==========
            csg = gcnp.tile([K, 1], f32)
            nc.sync.dma_start(csg[:], ccp_out)
            FTs = constp.tile([D, K], f32)
            nc.sync.dma_start(FTs[:], FT)
            Wd1s = constp.tile([D, HD], f32)
            nc.sync.dma_start(Wd1s[:], Wd1)
            bd1s = constp.tile([128, 2], f32)
            nc.sync.dma_start(bd1s[:], bd1)
            p1T = gcnp.tile([128, 2, K], bf16)
            for m in range(2):
                pp = psGen.tile([128, K], f32, tag="gen", name=f"p1t{m}")
                nc.tensor.matmul(pp[:], Wd1s[:, m * 128:(m + 1) * 128], FTs[:],
                                 start=True, stop=True)
                nc.vector.tensor_scalar(p1T[:, m, :], pp[:],
                                        bd1s[:, m:m + 1], 0.0,
                                        op0=mybir.AluOpType.add,
                                        op1=mybir.AluOpType.max)
            NWC = 4
            WC = G_SC // NWC
            for j in range(NWC):
                c0, c1 = j * WC, (j + 1) * WC
                pq = psGen.tile([K, WC], f32, tag="gen", name=f"prof{j}")
                wa = wd2p.tile([128, WC], bf16, tag="wd2", name=f"wa{j}")
                nc.sync.dma_start(wa[:], Wd2b[0:128, c0:c1])
                nc.tensor.matmul(pq[:], p1T[:, 0, :], wa[:], start=True,
                                 stop=False)
                wb = wd2p.tile([128, WC], bf16, tag="wd2", name=f"wb{j}")
                nc.sync.dma_start(wb[:], Wd2b[128:256, c0:c1])
                nc.tensor.matmul(pq[:], p1T[:, 1, :], wb[:], start=False,
                                 stop=False)
                wc = wd2p.tile([1, WC], bf16, tag="wd2c", name=f"wc{j}")
                nc.sync.dma_start(wc[:], Wd2b[256:257, c0:c1])
                nc.tensor.matmul(pq[:], ones1[:], wc[:], start=False,
                                 stop=True)
                mt = mrecp.tile([K, WC], f32, tag="mt", name=f"mt{j}")
                nc.vector.tensor_scalar_mul(mt[:], pq[:], csg[:])
                nc.sync.dma_start(M_out[:, c0:c1], mt[:])
            nc.sync.dma_start(B_out, Bs[:])

            # ========== B12: outs 4, then hT (off the critical path) ======
            soft_outs(4)
            hT = gcnp.tile([D, SSH], f32)
            nc.vector.tensor_scalar_add(hT[:, :512], pc[:], b2s[:])
            nc.vector.tensor_scalar_add(hT[:, 512:], pd[:], b2s[:])
            nc.sync.dma_start(hT_out, hT[:])

    nc.compile()
    _NC_CACHE["nc"] = nc
    return nc


def _prep_in_maps(inputs):
    from concourse import mybir
    bf16 = mybir.dt.np(mybir.dt.bfloat16)

    z = np.asarray(inputs["z"], dtype=np.float32)
    ei = np.asarray(inputs["edge_index"]).astype(np.int64)
    W1 = np.asarray(inputs["W1"], dtype=np.float32)
    b1 = np.asarray(inputs["b1"], dtype=np.float32)
    W2 = np.asarray(inputs["W2"], dtype=np.float32)
    b2 = np.asarray(inputs["b2"], dtype=np.float32)
    U = np.asarray(inputs["U"], dtype=np.float32)
    V = np.asarray(inputs["V"], dtype=np.float32)
    F = np.asarray(inputs["F"], dtype=np.float32)
    Wd1 = np.asarray(inputs["Wd1"], dtype=np.float32)
    bd1 = np.asarray(inputs["bd1"], dtype=np.float32)
    Wd2 = np.asarray(inputs["Wd2"], dtype=np.float32)
    bd2 = np.asarray(inputs["bd2"], dtype=np.float32)

    row, col = ei[0], ei[1]
    deg = np.bincount(col, minlength=S).astype(np.float32) + 1.0
    dinv = deg ** -0.5
    AhT = np.zeros((S, S), dtype=np.float32)
    np.add.at(AhT, (row, col), (dinv[row] * dinv[col]).astype(np.float32))
    idx = np.arange(S)
    AhT[idx, idx] += 1.0 / deg

    W1b = np.ascontiguousarray(W1).astype(bf16)
    b1r = b1.reshape(HE, 1)
    b2r = b2.reshape(D, 1)
    FTm = np.ascontiguousarray(F.T)
    bd1r = np.ascontiguousarray(bd1.reshape(2, 128).T)
    Wd2b = np.concatenate([Wd2, bd2[None, :]], axis=0).astype(bf16)

    in_maps = []
    for cidx in range(NCORES):
        r0, r1 = cidx * SSH, (cidx + 1) * SSH
        v0, v1 = cidx * CSH, (cidx + 1) * CSH
        U_t = np.ascontiguousarray(
            U[r0:r1].reshape(NRT, RT, NCH, CCH).transpose(0, 2, 1, 3)
            .astype(np.float16))
        in_maps.append({
            "U_sh": U_t,
            "zT_sh": np.ascontiguousarray(z[r0:r1].T).astype(bf16),
            "W1": W1b,
            "b1": b1r,
            "AhT_sh": np.ascontiguousarray(AhT[:, r0:r1]).astype(bf16),
            "W2": W2,
            "b2": b2r,
            "V_sh": np.ascontiguousarray(V[v0:v1]),
            "FT": FTm,
            "Wd1": Wd1,
            "bd1": bd1r,
            "Wd2b": Wd2b,
        })
    return in_maps


def kernel(**inputs):
    global LAST_RESULT
    from concourse.bass_utils import run_bass_kernel_spmd

    nc = _build_nc()
    in_maps = _prep_in_maps(inputs)
    res = run_bass_kernel_spmd(nc, in_maps, core_ids=list(range(NCORES)))
    LAST_RESULT = res

    A = np.concatenate(
        [np.asarray(r["A_sh"], dtype=np.float32).reshape(SSH, C)
         for r in res.results], axis=0)
    B = np.concatenate(
        [np.asarray(r["B_sh"], dtype=np.float32)
         .reshape(RT, NVT, K)
         .transpose(1, 0, 2).reshape(CSH, K) for r in res.results], axis=0)
    h = np.concatenate([np.asarray(r["hT_sh"], dtype=np.float32).T
                        for r in res.results], axis=0)
    M_rec = np.asarray(res.results[0]["M_sh"], dtype=np.float32)
    F = np.asarray(inputs["F"], dtype=np.float32)
    return (A, B, h, M_rec, F)


# revision 16
# speedup vs baseline: 1.1680x; 1.1680x over previous
"""Trainium2 Bass kernel for nn_AlternativeIdeaModel (gnn_message_passing).

SPMD over 8 NeuronCores, row-sharding spots (625/core) and V rows (2500/core).

U softmax [5000, 20000] dominates (memory regime): host pre-tiles U into
chunk-contiguous DRAM [NRT, NCH, 125, 2500]; the 50MB in-stream and the 25MB
bf16 A out-stream both run on the GpSimd SWDGE queue (fans each DMA across all
16 SDMA engines; the HWDGE rings cap at ~130GB/s for these shapes). exp with
free-dim accumulate on ScalarE, reciprocal+scale on VectorE. Outs for tile t
are emitted after tile t+1's ins so the single SWDGE queue never head-of-line
blocks on the softmax barrier.

GCN uses a dense normalized adjacency (host-built from edge_index), bf16,
column-sharded [5000, 625] and SBUF-resident; per layer: local xw = x @ Wl,
AllGather (bf16), transposed aggregation matmuls on TensorE. V softmax +
TensorE ones-matmul column sum + AllReduce give C*p for M_rec; the decoder
runs replicated. Collectives are emitted between softmax tiles so their queue
position matches when their inputs are ready. All other DMA traffic lives on
the Sync HWDGE ring, ordered by expected readiness.
"""

import os
import sys

sys.path.insert(0, "/opt/trn_rl_repo")

import numpy as np

S, C, G, D, K, HE, HD = 5000, 20000, 2000, 32, 20, 64, 256
G_SC = 2000
NCORES = 8
SSH = S // NCORES          # 625
CSH = C // NCORES          # 2500
RT = 125
NRT = SSH // RT            # 5
CCH = 2500
NCH = C // CCH             # 8
QW = 5000                  # quarter width (2 chunks)
NQ = C // QW               # 4 quarters per row tile
KT1 = (G + 127) // 128     # 16
KTA = (S + 127) // 128     # 40
NVT = CSH // RT            # 20

A_FP16 = True

_NC_CACHE = {}
LAST_RESULT = None


def _build_nc():
    if "nc" in _NC_CACHE:
        return _NC_CACHE["nc"]
    from concourse import bacc, tile, mybir

    f32 = mybir.dt.float32
    bf16 = mybir.dt.bfloat16
    fp16 = mybir.dt.float16
    dtA = fp16 if A_FP16 else f32
    dtU = fp16
    AF = mybir.ActivationFunctionType
    AX = mybir.AxisListType

    nc = bacc.Bacc("TRN2", target_bir_lowering=False, debug=False,
                   num_devices=NCORES)

    U = nc.dram_tensor("U_sh", [NRT, NCH, RT, CCH], dtU,
                       kind="ExternalInput").ap()
    zT = nc.dram_tensor("zT_sh", [G, SSH], bf16, kind="ExternalInput").ap()
    W1 = nc.dram_tensor("W1", [G, HE], bf16, kind="ExternalInput").ap()
    b1 = nc.dram_tensor("b1", [HE, 1], f32, kind="ExternalInput").ap()
    Ah = nc.dram_tensor("AhT_sh", [S, SSH], bf16, kind="ExternalInput").ap()
    W2 = nc.dram_tensor("W2", [HE, D], f32, kind="ExternalInput").ap()
    b2 = nc.dram_tensor("b2", [D, 1], f32, kind="ExternalInput").ap()
    V = nc.dram_tensor("V_sh", [CSH, K], f32, kind="ExternalInput").ap()
    FT = nc.dram_tensor("FT", [D, K], f32, kind="ExternalInput").ap()
    Wd1 = nc.dram_tensor("Wd1", [D, HD], f32, kind="ExternalInput").ap()
    bd1 = nc.dram_tensor("bd1", [128, 2], f32, kind="ExternalInput").ap()
    Wd2b = nc.dram_tensor("Wd2b", [HD + 1, G_SC], bf16,
                          kind="ExternalInput").ap()

    A_out = nc.dram_tensor("A_sh", [NRT, RT, C], dtA,
                           kind="ExternalOutput").ap()
    B_out = nc.dram_tensor("B_sh", [RT, NVT, K], f32,
                           kind="ExternalOutput").ap()
    hT_out = nc.dram_tensor("hT_sh", [D, SSH], f32, kind="ExternalOutput").ap()
    M_out = nc.dram_tensor("M_sh", [K, G_SC], f32, kind="ExternalOutput").ap()

    RG = [list(range(NCORES))]

    with tile.TileContext(nc) as tc:
        with (
            tc.tile_pool(name="const", bufs=1) as constp,
            tc.tile_pool(name="ahat", bufs=KTA) as ahatp,
            tc.tile_pool(name="gcn", bufs=1) as gcnp,
            tc.tile_pool(name="zt", bufs=6) as ztp,
            tc.tile_pool(name="wd2", bufs=2) as wd2p,
            tc.tile_pool(name="uin", bufs=7) as uinp,
            tc.tile_pool(name="ebuf", bufs=7) as ebufp,
            tc.tile_pool(name="acc", bufs=2) as accp,
            tc.tile_pool(name="mrec", bufs=1) as mrecp,
            tc.tile_pool(name="psAgg", bufs=1, space="PSUM") as psAgg,
            tc.tile_pool(name="psAggB", bufs=1, space="PSUM") as psAggB,
            tc.tile_pool(name="psGen", bufs=5, space="PSUM") as psGen,
            tc.tile_pool(name="dram", bufs=1, space="DRAM") as dramp,
        ):
            # ---------- helpers for the softmax tiles ----------
            tiles_state = {}

            def soft_ins(t, mid=None):
                accs = accp.tile([RT, NCH], f32, tag="accs", name=f"acc{t}")
                eq = [ebufp.tile([RT, QW], dtA, tag="eq", name=f"e{t}_{q}")
                      for q in range(NQ)]
                for c in range(NCH):
                    if c == NCH // 2 and mid is not None:
                        mid()
                    uc = uinp.tile([RT, CCH], dtU, tag="uc", name=f"u{t}_{c}")
                    nc.gpsimd.dma_start(uc[:], U[t, c])
                    q, o = divmod(c, NCH // NQ)
                    nc.scalar.activation(eq[q][:, o * CCH:(o + 1) * CCH],
                                         uc[:], AF.Exp,
                                         accum_out=accs[:, c:c + 1])
                ssum = accp.tile([RT, 1], f32, tag="ss", name=f"s{t}")
                nc.vector.reduce_sum(ssum[:], accs[:], axis=AX.X)
                rrec = accp.tile([RT, 1], f32, tag="rr", name=f"r{t}")
                nc.vector.reciprocal(rrec[:], ssum[:])
                tiles_state[t] = (eq, rrec)

            def soft_outs(t):
                eq, rrec = tiles_state.pop(t)
                for q in range(NQ):
                    nc.vector.tensor_scalar_mul(eq[q][:], eq[q][:], rrec[:])
                    nc.gpsimd.dma_start(A_out[t, :, q * QW:(q + 1) * QW],
                                        eq[q][:])

            # ========== B1: layer-1 local matmul path + pure loads =====
            W1s = constp.tile([128, KT1, HE], bf16)
            nc.sync.dma_start(
                W1s[:, :KT1 - 1, :],
                W1[: (KT1 - 1) * 128].rearrange("(k p) n -> p k n", p=128))
            nc.sync.dma_start(W1s[: G - (KT1 - 1) * 128, KT1 - 1, :],
                              W1[(KT1 - 1) * 128:])
            b1s = constp.tile([HE, 1], f32)
            nc.sync.dma_start(b1s[:], b1)
            ones125 = constp.tile([128, 1], f32)
            nc.vector.memset(ones125[:], 1.0)
            ones1 = constp.tile([1, K], bf16)
            nc.vector.memset(ones1[:], 1.0)

            xw1_ps = [psGen.tile([RT, HE], f32, tag="gen", name=f"psxw1_{i}")
                      for i in range(NRT)]
            for k in range(KT1):
                kk = 128 if k < KT1 - 1 else G - (KT1 - 1) * 128
                zt = ztp.tile([128, SSH], bf16, tag="zt", name=f"zt{k}")
                nc.sync.dma_start(zt[:kk, :], zT[k * 128:k * 128 + kk, :])
                for i in range(NRT):
                    nc.tensor.matmul(xw1_ps[i][:],
                                     zt[:kk, i * RT:(i + 1) * RT],
                                     W1s[:kk, k, :],
                                     start=(k == 0), stop=(k == KT1 - 1))
            xw1o = gcnp.tile([RT, NRT, HE], bf16)
            for i in range(NRT):
                nc.vector.tensor_copy(xw1o[:, i, :], xw1_ps[i][:])
            cc1_in = dramp.tile([SSH, HE], bf16)
            cc1_out = dramp.tile([S, HE], bf16)
            nc.sync.dma_start(cc1_in.rearrange("(i p) n -> p i n", p=RT),
                              xw1o[:])
            W2s = constp.tile([HE, D], f32)
            nc.sync.dma_start(W2s[:], W2)
            b2s = constp.tile([D, 1], f32)
            nc.sync.dma_start(b2s[:], b2)
            Vs = gcnp.tile([RT, NVT, K], f32)
            nc.sync.dma_start(Vs[:], V.rearrange("(n p) c -> p n c", p=RT))
            # ========== B2: softmax tile 0 (cc1 trigger mid-tile) ==========
            soft_ins(0, mid=lambda: nc.gpsimd.collective_compute(
                "AllGather", mybir.AluOpType.bypass, replica_groups=RG,
                ins=[cc1_in.opt()], outs=[cc1_out.opt()]))

            # AhatT tiles: pure loads on the fast SWDGE stream (keeps the
            # HWDGE ring free for the latency-critical GCN plumbing)
            ah_tiles = []
            for k in range(KTA):
                r0, r1 = k * 128, min(S, (k + 1) * 128)
                t_ = ahatp.tile([128, SSH], bf16, tag="ah", name=f"ah{k}")
                nc.gpsimd.dma_start(t_[: r1 - r0, :], Ah[r0:r1, :])
                ah_tiles.append(t_)

            # ========== B3: xw1_full readback ==========
            xw1f = gcnp.tile([128, KTA, HE], bf16)
            nc.sync.dma_start(
                xw1f[:, :KTA - 1, :],
                cc1_out[: (KTA - 1) * 128].rearrange("(k p) n -> p k n", p=128))
            nc.sync.dma_start(xw1f[: S - (KTA - 1) * 128, KTA - 1, :],
                              cc1_out[(KTA - 1) * 128:])

            # ========== B4: softmax tile 1, outs 0 ==========
            soft_ins(1)
            soft_outs(0)

            # ========== B5: V softmax + colsum ==========
            Bs = gcnp.tile([RT, NVT, K], f32)
            vsum = gcnp.tile([RT, NVT], f32)
            for n in range(NVT):
                nc.scalar.activation(Bs[:, n, :], Vs[:, n, :], AF.Exp,
                                     accum_out=vsum[:, n:n + 1])
            vrec = gcnp.tile([RT, NVT], f32)
            nc.vector.reciprocal(vrec[:], vsum[:])
            for n in range(NVT):
                nc.vector.tensor_scalar_mul(Bs[:, n, :], Bs[:, n, :],
                                            vrec[:, n:n + 1])
            cs_ps = psGen.tile([K, 1], f32, tag="gen", name="cs_ps")
            for n in range(NVT):
                nc.tensor.matmul(cs_ps[:], Bs[:, n, :], ones125[:RT, :],
                                 start=(n == 0), stop=(n == NVT - 1))
            cs_sb = gcnp.tile([K, 1], f32)
            nc.vector.tensor_copy(cs_sb[:], cs_ps[:])

            # ========== B6: softmax tile 2, outs 1 ==========
            soft_ins(2)
            soft_outs(1)

            # ========== B7: agg1 + h1 + xw2 + cc2 + ccp ==========
            pa = psAgg.tile([HE, 512], f32, tag="agga", name="pa1")
            pb = psAggB.tile([HE, SSH - 512], f32, tag="aggb", name="pb1")
            for k in range(KTA):
                kk = 128 if k < KTA - 1 else S - (KTA - 1) * 128
                nc.tensor.matmul(pa[:], xw1f[:kk, k, :], ah_tiles[k][:kk, :512],
                                 start=(k == 0), stop=(k == KTA - 1))
                nc.tensor.matmul(pb[:], xw1f[:kk, k, :], ah_tiles[k][:kk, 512:],
                                 start=(k == 0), stop=(k == KTA - 1))
            # h1T relu+bias on ScalarE — sits after tile-2 exps in the ACT
            # FIFO, matching when agg1 is actually ready
            h1T = gcnp.tile([HE, SSH], f32)
            nc.scalar.activation(h1T[:, :512], pa[:], AF.Relu, bias=b1s[:])
            nc.scalar.activation(h1T[:, 512:], pb[:], AF.Relu, bias=b1s[:])
            xw2o = gcnp.tile([RT, NRT, D], bf16)
            for i in range(NRT):
                ps = psGen.tile([RT, D], f32, tag="gen", name=f"psxw2_{i}")
                nc.tensor.matmul(ps[:], h1T[:, i * RT:(i + 1) * RT], W2s[:],
                                 start=True, stop=True)
                nc.vector.tensor_copy(xw2o[:, i, :], ps[:])
            cc2_in = dramp.tile([SSH, D], bf16)
            cc2_out = dramp.tile([S, D], bf16)
            nc.sync.dma_start(cc2_in.rearrange("(i p) n -> p i n", p=RT),
                              xw2o[:])
            nc.gpsimd.collective_compute(
                "AllGather", mybir.AluOpType.bypass, replica_groups=RG,
                ins=[cc2_in.opt()], outs=[cc2_out.opt()])
            ccp_in = dramp.tile([K, 1], f32)
            ccp_out = dramp.tile([K, 1], f32)
            nc.gpsimd.dma_start(ccp_in, cs_sb[:])
            nc.gpsimd.collective_compute(
                "AllReduce", mybir.AluOpType.add, replica_groups=RG,
                ins=[ccp_in.opt()], outs=[ccp_out.opt()])
            xw2f = gcnp.tile([128, KTA, D], bf16)
            nc.sync.dma_start(
                xw2f[:, :KTA - 1, :],
                cc2_out[: (KTA - 1) * 128].rearrange("(k p) n -> p k n", p=128))
            nc.sync.dma_start(xw2f[: S - (KTA - 1) * 128, KTA - 1, :],
                              cc2_out[(KTA - 1) * 128:])

            # ========== B8: softmax tile 3, outs 2 ==========
            soft_ins(3)
            soft_outs(2)

            # ========== B9: agg2 matmuls ==========
            pc = psAgg.tile([D, 512], f32, tag="agga", name="pa2")
            pd = psAggB.tile([D, SSH - 512], f32, tag="aggb", name="pb2")
            for k in range(KTA):
                kk = 128 if k < KTA - 1 else S - (KTA - 1) * 128
                nc.tensor.matmul(pc[:], xw2f[:kk, k, :], ah_tiles[k][:kk, :512],
                                 start=(k == 0), stop=(k == KTA - 1))
                nc.tensor.matmul(pd[:], xw2f[:kk, k, :], ah_tiles[k][:kk, 512:],
                                 start=(k == 0), stop=(k == KTA - 1))

            # ========== B10: softmax tile 4, outs 3 ==========
            soft_ins(4)
            soft_outs(3)

            # ========== B11: decoder + M_rec ==========
            csg = gcnp.tile([K, 1], f32)
            nc.sync.dma_start(csg[:], ccp_out)
            FTs = constp.tile([D, K], f32)
            nc.sync.dma_start(FTs[:], FT)
            Wd1s = constp.tile([D, HD], f32)
            nc.sync.dma_start(Wd1s[:], Wd1)
            bd1s = constp.tile([128, 2], f32)
            nc.sync.dma_start(bd1s[:], bd1)
            p1T = gcnp.tile([128, 2, K], bf16)
            for m in range(2):
                pp = psGen.tile([128, K], f32, tag="gen", name=f"p1t{m}")
                nc.tensor.matmul(pp[:], Wd1s[:, m * 128:(m + 1) * 128], FTs[:],
                                 start=True, stop=True)
                nc.vector.tensor_scalar(p1T[:, m, :], pp[:],
                                        bd1s[:, m:m + 1], 0.0,
                                        op0=mybir.AluOpType.add,
                                        op1=mybir.AluOpType.max)
            NWC = 4
            WC = G_SC // NWC
            for j in range(NWC):
                c0, c1 = j * WC, (j + 1) * WC
                pq = psGen.tile([K, WC], f32, tag="gen", name=f"prof{j}")
                wa = wd2p.tile([128, WC], bf16, tag="wd2", name=f"wa{j}")
                nc.sync.dma_start(wa[:], Wd2b[0:128, c0:c1])
                nc.tensor.matmul(pq[:], p1T[:, 0, :], wa[:], start=True,
                                 stop=False)
                wb = wd2p.tile([128, WC], bf16, tag="wd2", name=f"wb{j}")
                nc.sync.dma_start(wb[:], Wd2b[128:256, c0:c1])
                nc.tensor.matmul(pq[:], p1T[:, 1, :], wb[:], start=False,
                                 stop=False)
                wc = wd2p.tile([1, WC], bf16, tag="wd2c", name=f"wc{j}")
                nc.sync.dma_start(wc[:], Wd2b[256:257, c0:c1])
                nc.tensor.matmul(pq[:], ones1[:], wc[:], start=False,
                                 stop=True)
                mt = mrecp.tile([K, WC], f32, tag="mt", name=f"mt{j}")
                nc.vector.tensor_scalar_mul(mt[:], pq[:], csg[:])
                nc.sync.dma_start(M_out[:, c0:c1], mt[:])
            nc.sync.dma_start(B_out, Bs[:])

            # ========== B12: outs 4, then hT (off the critical path) ======
            soft_outs(4)
            hT = gcnp.tile([D, SSH], f32)
            nc.vector.tensor_scalar_add(hT[:, :512], pc[:], b2s[:])
            nc.vector.tensor_scalar_add(hT[:, 512:], pd[:], b2s[:])
            nc.sync.dma_start(hT_out, hT[:])

    nc.compile()
    _NC_CACHE["nc"] = nc
    return nc


def _prep_in_maps(inputs):
    from concourse import mybir
    bf16 = mybir.dt.np(mybir.dt.bfloat16)

    z = np.asarray(inputs["z"], dtype=np.float32)
    ei = np.asarray(inputs["edge_index"]).astype(np.int64)
    W1 = np.asarray(inputs["W1"], dtype=np.float32)
    b1 = np.asarray(inputs["b1"], dtype=np.float32)
    W2 = np.asarray(inputs["W2"], dtype=np.float32)
    b2 = np.asarray(inputs["b2"], dtype=np.float32)
    U = np.asarray(inputs["U"], dtype=np.float32)
    V = np.asarray(inputs["V"], dtype=np.float32)
    F = np.asarray(inputs["F"], dtype=np.float32)
    Wd1 = np.asarray(inputs["Wd1"], dtype=np.float32)
    bd1 = np.asarray(inputs["bd1"], dtype=np.float32)
    Wd2 = np.asarray(inputs["Wd2"], dtype=np.float32)
    bd2 = np.asarray(inputs["bd2"], dtype=np.float32)

    row, col = ei[0], ei[1]
    deg = np.bincount(col, minlength=S).astype(np.float32) + 1.0
    dinv = deg ** -0.5
    AhT = np.zeros((S, S), dtype=np.float32)
    np.add.at(AhT, (row, col), (dinv[row] * dinv[col]).astype(np.float32))
    idx = np.arange(S)
    AhT[idx, idx] += 1.0 / deg

    W1b = np.ascontiguousarray(W1).astype(bf16)
    b1r = b1.reshape(HE, 1)
    b2r = b2.reshape(D, 1)
    FTm = np.ascontiguousarray(F.T)
    bd1r = np.ascontiguousarray(bd1.reshape(2, 128).T)
    Wd2b = np.concatenate([Wd2, bd2[None, :]], axis=0).astype(bf16)

    in_maps = []
    for cidx in range(NCORES):
        r0, r1 = cidx * SSH, (cidx + 1) * SSH
        v0, v1 = cidx * CSH, (cidx + 1) * CSH
        U_t = np.ascontiguousarray(
            U[r0:r1].reshape(NRT, RT, NCH, CCH).transpose(0, 2, 1, 3)
            .astype(np.float16))
        in_maps.append({
            "U_sh": U_t,
            "zT_sh": np.ascontiguousarray(z[r0:r1].T).astype(bf16),
            "W1": W1b,
            "b1": b1r,
            "AhT_sh": np.ascontiguousarray(AhT[:, r0:r1]).astype(bf16),
            "W2": W2,
            "b2": b2r,
            "V_sh": np.ascontiguousarray(V[v0:v1]),
            "FT": FTm,
            "Wd1": Wd1,
            "bd1": bd1r,
            "Wd2b": Wd2b,
        })
    return in_maps


def kernel(**inputs):
    global LAST_RESULT
    from concourse.bass_utils import run_bass_kernel_spmd

    nc = _build_nc()
    in_maps = _prep_in_maps(inputs)
    res = run_bass_kernel_spmd(nc, in_maps, core_ids=list(range(NCORES)))
    LAST_RESULT = res

    A = np.concatenate(
        [np.asarray(r["A_sh"], dtype=np.float32).reshape(SSH, C)
         for r in res.results], axis=0)
    B = np.concatenate(
        [np.asarray(r["B_sh"], dtype=np.float32)
         .reshape(RT, NVT, K)
         .transpose(1, 0, 2).reshape(CSH, K) for r in res.results], axis=0)
    h = np.concatenate([np.asarray(r["hT_sh"], dtype=np.float32).T
                        for r in res.results], axis=0)
    M_rec = np.asarray(res.results[0]["M_sh"], dtype=np.float32)
    F = np.asarray(inputs["F"], dtype=np.float32)
    return (A, B, h, M_rec, F)


# revision 17
# speedup vs baseline: 1.2293x; 1.0525x over previous
"""Trainium2 Bass kernel for nn_AlternativeIdeaModel (gnn_message_passing).

SPMD over 8 NeuronCores, row-sharding spots (625/core) and V rows (2500/core).

U softmax [5000, 20000] dominates (memory regime): host pre-tiles U into
chunk-contiguous DRAM [NRT, NCH, 125, 2500]; the 50MB in-stream and the 25MB
bf16 A out-stream both run on the GpSimd SWDGE queue (fans each DMA across all
16 SDMA engines; the HWDGE rings cap at ~130GB/s for these shapes). exp with
free-dim accumulate on ScalarE, reciprocal+scale on VectorE. Outs for tile t
are emitted after tile t+1's ins so the single SWDGE queue never head-of-line
blocks on the softmax barrier.

GCN uses a dense normalized adjacency (host-built from edge_index), bf16,
column-sharded [5000, 625] and SBUF-resident; per layer: local xw = x @ Wl,
AllGather (bf16), transposed aggregation matmuls on TensorE. V softmax +
TensorE ones-matmul column sum + AllReduce give C*p for M_rec; the decoder
runs replicated. Collectives are emitted between softmax tiles so their queue
position matches when their inputs are ready. All other DMA traffic lives on
the Sync HWDGE ring, ordered by expected readiness.
"""

import os
import sys

sys.path.insert(0, "/opt/trn_rl_repo")

import numpy as np

S, C, G, D, K, HE, HD = 5000, 20000, 2000, 32, 20, 64, 256
G_SC = 2000
NCORES = 8
SSH = S // NCORES          # 625
CSH = C // NCORES          # 2500
RT = 125
NRT = SSH // RT            # 5
CCH = 2500
NCH = C // CCH             # 8
QW = 5000                  # quarter width (2 chunks)
NQ = C // QW               # 4 quarters per row tile
KT1 = (G + 127) // 128     # 16
KTA = (S + 127) // 128     # 40
NVT = CSH // RT            # 20

A_FP16 = True

_NC_CACHE = {}
LAST_RESULT = None


def _build_nc():
    if "nc" in _NC_CACHE:
        return _NC_CACHE["nc"]
    from concourse import bacc, tile, mybir

    f32 = mybir.dt.float32
    bf16 = mybir.dt.bfloat16
    fp16 = mybir.dt.float16
    dtA = fp16 if A_FP16 else f32
    dtU = fp16
    AF = mybir.ActivationFunctionType
    AX = mybir.AxisListType

    nc = bacc.Bacc("TRN2", target_bir_lowering=False, debug=False,
                   num_devices=NCORES)

    U = nc.dram_tensor("U_sh", [NRT, NCH, RT, CCH], dtU,
                       kind="ExternalInput").ap()
    zT = nc.dram_tensor("zT_sh", [G, SSH], bf16, kind="ExternalInput").ap()
    W1 = nc.dram_tensor("W1", [G, HE], bf16, kind="ExternalInput").ap()
    b1 = nc.dram_tensor("b1", [HE, 1], f32, kind="ExternalInput").ap()
    Ah = nc.dram_tensor("AhT_sh", [S, SSH], bf16, kind="ExternalInput").ap()
    W2 = nc.dram_tensor("W2", [HE, D], f32, kind="ExternalInput").ap()
    b2 = nc.dram_tensor("b2", [D, 1], f32, kind="ExternalInput").ap()
    V = nc.dram_tensor("V_sh", [CSH, K], f32, kind="ExternalInput").ap()
    FT = nc.dram_tensor("FT", [D, K], f32, kind="ExternalInput").ap()
    Wd1 = nc.dram_tensor("Wd1", [D, HD], f32, kind="ExternalInput").ap()
    bd1 = nc.dram_tensor("bd1", [128, 2], f32, kind="ExternalInput").ap()
    Wd2b = nc.dram_tensor("Wd2b", [HD + 1, G_SC], bf16,
                          kind="ExternalInput").ap()

    A_out = nc.dram_tensor("A_sh", [NRT, RT, C], dtA,
                           kind="ExternalOutput").ap()
    B_out = nc.dram_tensor("B_sh", [RT, NVT, K], f32,
                           kind="ExternalOutput").ap()
    hT_out = nc.dram_tensor("hT_sh", [D, SSH], f32, kind="ExternalOutput").ap()
    M_out = nc.dram_tensor("M_sh", [K, G_SC], f32, kind="ExternalOutput").ap()

    RG = [list(range(NCORES))]

    with tile.TileContext(nc) as tc:
        with (
            tc.tile_pool(name="const", bufs=1) as constp,
            tc.tile_pool(name="ahat", bufs=KTA) as ahatp,
            tc.tile_pool(name="gcn", bufs=1) as gcnp,
            tc.tile_pool(name="zt", bufs=6) as ztp,
            tc.tile_pool(name="wd2", bufs=2) as wd2p,
            tc.tile_pool(name="uin", bufs=5) as uinp,
            tc.tile_pool(name="ebuf", bufs=8) as ebufp,
            tc.tile_pool(name="acc", bufs=2) as accp,
            tc.tile_pool(name="mrec", bufs=1) as mrecp,
            tc.tile_pool(name="psAgg", bufs=1, space="PSUM") as psAgg,
            tc.tile_pool(name="psAggB", bufs=1, space="PSUM") as psAggB,
            tc.tile_pool(name="psGen", bufs=5, space="PSUM") as psGen,
            tc.tile_pool(name="dram", bufs=1, space="DRAM") as dramp,
        ):
            # ---------- helpers for the softmax tiles ----------
            tiles_state = {}

            def soft_ins(t, mid=None):
                accs = accp.tile([RT, NCH], f32, tag="accs", name=f"acc{t}")
                eq = [ebufp.tile([RT, QW], dtA, tag="eq", name=f"e{t}_{q}")
                      for q in range(NQ)]
                for c in range(NCH):
                    if c == NCH // 2 and mid is not None:
                        mid()
                    uc = uinp.tile([RT, CCH], dtU, tag="uc", name=f"u{t}_{c}")
                    nc.gpsimd.dma_start(uc[:], U[t, c])
                    q, o = divmod(c, NCH // NQ)
                    nc.scalar.activation(eq[q][:, o * CCH:(o + 1) * CCH],
                                         uc[:], AF.Exp,
                                         accum_out=accs[:, c:c + 1])
                ssum = accp.tile([RT, 1], f32, tag="ss", name=f"s{t}")
                nc.vector.reduce_sum(ssum[:], accs[:], axis=AX.X)
                rrec = accp.tile([RT, 1], f32, tag="rr", name=f"r{t}")
                nc.vector.reciprocal(rrec[:], ssum[:])
                tiles_state[t] = (eq, rrec)

            def soft_outs(t):
                eq, rrec = tiles_state.pop(t)
                for q in range(NQ):
                    nc.vector.tensor_scalar_mul(eq[q][:], eq[q][:], rrec[:])
                    nc.gpsimd.dma_start(A_out[t, :, q * QW:(q + 1) * QW],
                                        eq[q][:])

            # ========== B1: layer-1 local matmul path + pure loads =====
            W1s = constp.tile([128, KT1, HE], bf16)
            nc.sync.dma_start(
                W1s[:, :KT1 - 1, :],
                W1[: (KT1 - 1) * 128].rearrange("(k p) n -> p k n", p=128))
            nc.sync.dma_start(W1s[: G - (KT1 - 1) * 128, KT1 - 1, :],
                              W1[(KT1 - 1) * 128:])
            b1s = constp.tile([HE, 1], f32)
            nc.sync.dma_start(b1s[:], b1)
            ones125 = constp.tile([128, 1], f32)
            nc.vector.memset(ones125[:], 1.0)
            ones1 = constp.tile([1, K], bf16)
            nc.vector.memset(ones1[:], 1.0)

            xw1_ps = [psGen.tile([RT, HE], f32, tag="gen", name=f"psxw1_{i}")
                      for i in range(NRT)]
            for k in range(KT1):
                kk = 128 if k < KT1 - 1 else G - (KT1 - 1) * 128
                zt = ztp.tile([128, SSH], bf16, tag="zt", name=f"zt{k}")
                nc.sync.dma_start(zt[:kk, :], zT[k * 128:k * 128 + kk, :])
                for i in range(NRT):
                    nc.tensor.matmul(xw1_ps[i][:],
                                     zt[:kk, i * RT:(i + 1) * RT],
                                     W1s[:kk, k, :],
                                     start=(k == 0), stop=(k == KT1 - 1))
            xw1o = gcnp.tile([RT, NRT, HE], bf16)
            for i in range(NRT):
                nc.vector.tensor_copy(xw1o[:, i, :], xw1_ps[i][:])
            cc1_in = dramp.tile([SSH, HE], bf16)
            cc1_out = dramp.tile([S, HE], bf16)
            nc.sync.dma_start(cc1_in.rearrange("(i p) n -> p i n", p=RT),
                              xw1o[:])
            W2s = constp.tile([HE, D], f32)
            nc.sync.dma_start(W2s[:], W2)
            b2s = constp.tile([D, 1], f32)
            nc.sync.dma_start(b2s[:], b2)
            Vs = gcnp.tile([RT, NVT, K], f32)
            nc.sync.dma_start(Vs[:], V.rearrange("(n p) c -> p n c", p=RT))
            # ========== B2: softmax tile 0 (cc1 trigger mid-tile) ==========
            soft_ins(0, mid=lambda: nc.gpsimd.collective_compute(
                "AllGather", mybir.AluOpType.bypass, replica_groups=RG,
                ins=[cc1_in.opt()], outs=[cc1_out.opt()]))

            # AhatT tiles: pure loads on the fast SWDGE stream (keeps the
            # HWDGE ring free for the latency-critical GCN plumbing)
            ah_tiles = []
            for k in range(KTA):
                r0, r1 = k * 128, min(S, (k + 1) * 128)
                t_ = ahatp.tile([128, SSH], bf16, tag="ah", name=f"ah{k}")
                nc.gpsimd.dma_start(t_[: r1 - r0, :], Ah[r0:r1, :])
                ah_tiles.append(t_)

            # ========== B3: xw1_full readback ==========
            xw1f = gcnp.tile([128, KTA, HE], bf16)
            nc.sync.dma_start(
                xw1f[:, :KTA - 1, :],
                cc1_out[: (KTA - 1) * 128].rearrange("(k p) n -> p k n", p=128))
            nc.sync.dma_start(xw1f[: S - (KTA - 1) * 128, KTA - 1, :],
                              cc1_out[(KTA - 1) * 128:])

            # ========== B4: softmax tile 1, outs 0 ==========
            soft_ins(1)
            soft_outs(0)

            # ========== B5: V softmax + colsum ==========
            Bs = gcnp.tile([RT, NVT, K], f32)
            vsum = gcnp.tile([RT, NVT], f32)
            for n in range(NVT):
                nc.scalar.activation(Bs[:, n, :], Vs[:, n, :], AF.Exp,
                                     accum_out=vsum[:, n:n + 1])
            vrec = gcnp.tile([RT, NVT], f32)
            nc.vector.reciprocal(vrec[:], vsum[:])
            for n in range(NVT):
                nc.vector.tensor_scalar_mul(Bs[:, n, :], Bs[:, n, :],
                                            vrec[:, n:n + 1])
            cs_ps = psGen.tile([K, 1], f32, tag="gen", name="cs_ps")
            for n in range(NVT):
                nc.tensor.matmul(cs_ps[:], Bs[:, n, :], ones125[:RT, :],
                                 start=(n == 0), stop=(n == NVT - 1))
            cs_sb = gcnp.tile([K, 1], f32)
            nc.vector.tensor_copy(cs_sb[:], cs_ps[:])

            # ========== B6: softmax tile 2, outs 1 ==========
            soft_ins(2)
            soft_outs(1)

            # ========== B7: agg1 + h1 + xw2 + cc2 + ccp ==========
            pa = psAgg.tile([HE, 512], f32, tag="agga", name="pa1")
            pb = psAggB.tile([HE, SSH - 512], f32, tag="aggb", name="pb1")
            for k in range(KTA):
                kk = 128 if k < KTA - 1 else S - (KTA - 1) * 128
                nc.tensor.matmul(pa[:], xw1f[:kk, k, :], ah_tiles[k][:kk, :512],
                                 start=(k == 0), stop=(k == KTA - 1))
                nc.tensor.matmul(pb[:], xw1f[:kk, k, :], ah_tiles[k][:kk, 512:],
                                 start=(k == 0), stop=(k == KTA - 1))
            # h1T relu+bias on ScalarE — sits after tile-2 exps in the ACT
            # FIFO, matching when agg1 is actually ready
            h1T = gcnp.tile([HE, SSH], f32)
            nc.scalar.activation(h1T[:, :512], pa[:], AF.Relu, bias=b1s[:])
            nc.scalar.activation(h1T[:, 512:], pb[:], AF.Relu, bias=b1s[:])
            xw2o = gcnp.tile([RT, NRT, D], bf16)
            for i in range(NRT):
                ps = psGen.tile([RT, D], f32, tag="gen", name=f"psxw2_{i}")
                nc.tensor.matmul(ps[:], h1T[:, i * RT:(i + 1) * RT], W2s[:],
                                 start=True, stop=True)
                nc.vector.tensor_copy(xw2o[:, i, :], ps[:])
            cc2_in = dramp.tile([SSH, D], bf16)
            cc2_out = dramp.tile([S, D], bf16)
            nc.sync.dma_start(cc2_in.rearrange("(i p) n -> p i n", p=RT),
                              xw2o[:])
            nc.gpsimd.collective_compute(
                "AllGather", mybir.AluOpType.bypass, replica_groups=RG,
                ins=[cc2_in.opt()], outs=[cc2_out.opt()])
            ccp_in = dramp.tile([K, 1], f32)
            ccp_out = dramp.tile([K, 1], f32)
            nc.gpsimd.dma_start(ccp_in, cs_sb[:])
            nc.gpsimd.collective_compute(
                "AllReduce", mybir.AluOpType.add, replica_groups=RG,
                ins=[ccp_in.opt()], outs=[ccp_out.opt()])
            xw2f = gcnp.tile([128, KTA, D], bf16)
            nc.sync.dma_start(
                xw2f[:, :KTA - 1, :],
                cc2_out[: (KTA - 1) * 128].rearrange("(k p) n -> p k n", p=128))
            nc.sync.dma_start(xw2f[: S - (KTA - 1) * 128, KTA - 1, :],
                              cc2_out[(KTA - 1) * 128:])

            # ========== B8: softmax tile 3, outs 2 ==========
            soft_ins(3)
            soft_outs(2)

            # ========== B9: agg2 matmuls ==========
            pc = psAgg.tile([D, 512], f32, tag="agga", name="pa2")
            pd = psAggB.tile([D, SSH - 512], f32, tag="aggb", name="pb2")
            for k in range(KTA):
                kk = 128 if k < KTA - 1 else S - (KTA - 1) * 128
                nc.tensor.matmul(pc[:], xw2f[:kk, k, :], ah_tiles[k][:kk, :512],
                                 start=(k == 0), stop=(k == KTA - 1))
                nc.tensor.matmul(pd[:], xw2f[:kk, k, :], ah_tiles[k][:kk, 512:],
                                 start=(k == 0), stop=(k == KTA - 1))

            # ========== B10: softmax tile 4, outs 3 ==========
            soft_ins(4)
            soft_outs(3)

            # ========== B11: decoder + M_rec ==========
            csg = gcnp.tile([K, 1], f32)
            nc.sync.dma_start(csg[:], ccp_out)
            FTs = constp.tile([D, K], f32)
            nc.sync.dma_start(FTs[:], FT)
            Wd1s = constp.tile([D, HD], f32)
            nc.sync.dma_start(Wd1s[:], Wd1)
            bd1s = constp.tile([128, 2], f32)
            nc.sync.dma_start(bd1s[:], bd1)
            p1T = gcnp.tile([128, 2, K], bf16)
            for m in range(2):
                pp = psGen.tile([128, K], f32, tag="gen", name=f"p1t{m}")
                nc.tensor.matmul(pp[:], Wd1s[:, m * 128:(m + 1) * 128], FTs[:],
                                 start=True, stop=True)
                nc.vector.tensor_scalar(p1T[:, m, :], pp[:],
                                        bd1s[:, m:m + 1], 0.0,
                                        op0=mybir.AluOpType.add,
                                        op1=mybir.AluOpType.max)
            NWC = 4
            WC = G_SC // NWC
            for j in range(NWC):
                c0, c1 = j * WC, (j + 1) * WC
                pq = psGen.tile([K, WC], f32, tag="gen", name=f"prof{j}")
                wa = wd2p.tile([128, WC], bf16, tag="wd2", name=f"wa{j}")
                nc.sync.dma_start(wa[:], Wd2b[0:128, c0:c1])
                nc.tensor.matmul(pq[:], p1T[:, 0, :], wa[:], start=True,
                                 stop=False)
                wb = wd2p.tile([128, WC], bf16, tag="wd2", name=f"wb{j}")
                nc.sync.dma_start(wb[:], Wd2b[128:256, c0:c1])
                nc.tensor.matmul(pq[:], p1T[:, 1, :], wb[:], start=False,
                                 stop=False)
                wc = wd2p.tile([1, WC], bf16, tag="wd2c", name=f"wc{j}")
                nc.sync.dma_start(wc[:], Wd2b[256:257, c0:c1])
                nc.tensor.matmul(pq[:], ones1[:], wc[:], start=False,
                                 stop=True)
                mt = mrecp.tile([K, WC], f32, tag="mt", name=f"mt{j}")
                nc.vector.tensor_scalar_mul(mt[:], pq[:], csg[:])
                nc.sync.dma_start(M_out[:, c0:c1], mt[:])
            nc.sync.dma_start(B_out, Bs[:])

            # ========== B12: outs 4, then hT (off the critical path) ======
            soft_outs(4)
            hT = gcnp.tile([D, SSH], f32)
            nc.vector.tensor_scalar_add(hT[:, :512], pc[:], b2s[:])
            nc.vector.tensor_scalar_add(hT[:, 512:], pd[:], b2s[:])
            nc.sync.dma_start(hT_out, hT[:])

    nc.compile()
    _NC_CACHE["nc"] = nc
    return nc


def _prep_in_maps(inputs):
    from concourse import mybir
    bf16 = mybir.dt.np(mybir.dt.bfloat16)

    z = np.asarray(inputs["z"], dtype=np.float32)
    ei = np.asarray(inputs["edge_index"]).astype(np.int64)
    W1 = np.asarray(inputs["W1"], dtype=np.float32)
    b1 = np.asarray(inputs["b1"], dtype=np.float32)
    W2 = np.asarray(inputs["W2"], dtype=np.float32)
    b2 = np.asarray(inputs["b2"], dtype=np.float32)
    U = np.asarray(inputs["U"], dtype=np.float32)
    V = np.asarray(inputs["V"], dtype=np.float32)
    F = np.asarray(inputs["F"], dtype=np.float32)
    Wd1 = np.asarray(inputs["Wd1"], dtype=np.float32)
    bd1 = np.asarray(inputs["bd1"], dtype=np.float32)
    Wd2 = np.asarray(inputs["Wd2"], dtype=np.float32)
    bd2 = np.asarray(inputs["bd2"], dtype=np.float32)

    row, col = ei[0], ei[1]
    deg = np.bincount(col, minlength=S).astype(np.float32) + 1.0
    dinv = deg ** -0.5
    AhT = np.zeros((S, S), dtype=np.float32)
    np.add.at(AhT, (row, col), (dinv[row] * dinv[col]).astype(np.float32))
    idx = np.arange(S)
    AhT[idx, idx] += 1.0 / deg

    W1b = np.ascontiguousarray(W1).astype(bf16)
    b1r = b1.reshape(HE, 1)
    b2r = b2.reshape(D, 1)
    FTm = np.ascontiguousarray(F.T)
    bd1r = np.ascontiguousarray(bd1.reshape(2, 128).T)
    Wd2b = np.concatenate([Wd2, bd2[None, :]], axis=0).astype(bf16)

    in_maps = []
    for cidx in range(NCORES):
        r0, r1 = cidx * SSH, (cidx + 1) * SSH
        v0, v1 = cidx * CSH, (cidx + 1) * CSH
        U_t = np.ascontiguousarray(
            U[r0:r1].reshape(NRT, RT, NCH, CCH).transpose(0, 2, 1, 3)
            .astype(np.float16))
        in_maps.append({
            "U_sh": U_t,
            "zT_sh": np.ascontiguousarray(z[r0:r1].T).astype(bf16),
            "W1": W1b,
            "b1": b1r,
            "AhT_sh": np.ascontiguousarray(AhT[:, r0:r1]).astype(bf16),
            "W2": W2,
            "b2": b2r,
            "V_sh": np.ascontiguousarray(V[v0:v1]),
            "FT": FTm,
            "Wd1": Wd1,
            "bd1": bd1r,
            "Wd2b": Wd2b,
        })
    return in_maps


def kernel(**inputs):
    global LAST_RESULT
    from concourse.bass_utils import run_bass_kernel_spmd

    nc = _build_nc()
    in_maps = _prep_in_maps(inputs)
    res = run_bass_kernel_spmd(nc, in_maps, core_ids=list(range(NCORES)))
    LAST_RESULT = res

    A = np.concatenate(
        [np.asarray(r["A_sh"], dtype=np.float32).reshape(SSH, C)
         for r in res.results], axis=0)
    B = np.concatenate(
        [np.asarray(r["B_sh"], dtype=np.float32)
         .reshape(RT, NVT, K)
         .transpose(1, 0, 2).reshape(CSH, K) for r in res.results], axis=0)
    h = np.concatenate([np.asarray(r["hT_sh"], dtype=np.float32).T
                        for r in res.results], axis=0)
    M_rec = np.asarray(res.results[0]["M_sh"], dtype=np.float32)
    F = np.asarray(inputs["F"], dtype=np.float32)
    return (A, B, h, M_rec, F)
